# revision 25
# baseline (speedup 1.0000x reference)
"""GCN layer on 8 trn2 cores.

Math: out = segment_sum((h@W * norm)[src], dst) * norm + bias
Linearity reorder: out = (segment_sum((h*norm)[src], dst) @ W) * norm + bias
=> aggregate input features first (partitioned by dst), GEMM + epilogue per
   dst shard afterwards.

Host->device traffic is the bottleneck (axon tunnel ~40-60MB/s), so:
- each core uploads only its 1/8 shard of (h*norm) int8-quantized with
  exactly-invertible per-row f32 scales, plus its 1/8 of W in bf16; full
  tables are assembled on-device with AllGathers over NeuronLink
- edge src ids and dst-slot ids go up packed in one int16 tensor
- the output returns int8-quantized (offset-128 uint8) with per-row f32
  reciprocal scales; the dst-norm multiply and bias add fold exactly into
  the host-side dequant
- constants (iota/identity) are generated on-device
- jax persistent compilation cache (keyed per kernel-source hash to avoid
  stale cross-version NEFF collisions) avoids per-process recompiles
"""
import os
import hashlib
import numpy as np
from contextlib import ExitStack

import jax
with open(__file__, "rb") as _f:
    _SRC_HASH = hashlib.sha256(_f.read()).hexdigest()[:16]
jax.config.update("jax_compilation_cache_dir",
                  os.environ.get("KERNEL_JAX_CACHE",
                                 f"/tmp/jax_cache_gcn_{_SRC_HASH}"))
jax.config.update("jax_persistent_cache_min_compile_time_secs", 0)
jax.config.update("jax_persistent_cache_min_entry_size_bytes", 0)

import concourse.bass as bass
import concourse.bacc as bacc
import concourse.mybir as mybir
import concourse.tile as tile
from concourse.masks import make_identity
from concourse.bass_utils import run_bass_kernel_spmd

P = 128
N = 10000
D = 512
NCORES = 8
NPAD = 10240            # N padded to multiple of 128*NCORES
NPC = NPAD // NCORES    # node rows per core = 1280
WPC = D // NCORES       # weight rows per core = 64
SROWS = NPC + WPC       # uploaded rows per core (nodes + W slice) = 1344
GROWS = SROWS * NCORES  # gathered rows = 10752
NBLK = NPC // P         # dst blocks per core = 10
KC = D // P             # feature chunks = 4


def _build(C):
    """Build the single SPMD Bass program. C = edge chunks per dst block."""
    nc = bacc.Bacc(None, target_bir_lowering=False)
    f32 = mybir.dt.float32
    bf16 = mybir.dt.bfloat16
    i32 = mybir.dt.int32
    i16 = mybir.dt.int16
    i8 = mybir.dt.int8
    u8 = mybir.dt.uint8

    hq = nc.declare_dram_parameter("hq", [SROWS, D], i8, isOutput=False)
    hs = nc.declare_dram_parameter("hs", [SROWS, 1], f32, isOutput=False)
    edg = nc.declare_dram_parameter("edg", [NBLK, P, 2 * C], i16, isOutput=False)
    out = nc.declare_dram_parameter("out", [NPC, D], u8, isOutput=True)
    osc = nc.declare_dram_parameter("osc", [NPC, 1], f32, isOutput=True)

    with tile.TileContext(nc) as tc, ExitStack() as ctx:
        dram = ctx.enter_context(tc.tile_pool(name="dram", bufs=6, space="DRAM"))
        const = ctx.enter_context(tc.tile_pool(name="const", bufs=1))
        epool = ctx.enter_context(tc.tile_pool(name="edges", bufs=NBLK))
        gpool = ctx.enter_context(tc.tile_pool(name="gath", bufs=8))
        spool = ctx.enter_context(tc.tile_pool(name="sel", bufs=8))
        apool = ctx.enter_context(tc.tile_pool(name="accs", bufs=NBLK))
        tpool = ctx.enter_context(tc.tile_pool(name="trs", bufs=4 * NBLK))
        opool = ctx.enter_context(tc.tile_pool(name="outs", bufs=2 * NBLK))
        ps1 = ctx.enter_context(tc.tile_pool(name="ps1", bufs=2, space="PSUM"))
        pst = ctx.enter_context(tc.tile_pool(name="pst", bufs=4, space="PSUM"))
        ps2 = ctx.enter_context(tc.tile_pool(name="ps2", bufs=2, space="PSUM"))

        # Assemble full tables on-device: each core uploads its 1/8 of the
        # int8 node rows plus its 1/8 of int8 W rows (and their f32 scales);
        # AllGather moves the rest over NeuronLink.
        hqb = dram.tile([SROWS, D], i8)
        hq_gat = dram.tile([GROWS, D], i8)
        nc.gpsimd.dma_start(out=hqb[:], in_=hq[:])
        nc.gpsimd.collective_compute(
            "AllGather", mybir.AluOpType.bypass,
            replica_groups=[list(range(NCORES))],
            ins=[hqb.opt()], outs=[hq_gat.opt()])

        hsb = dram.tile([SROWS, 1], f32)
        hs_gat = dram.tile([GROWS, 1], f32)
        nc.gpsimd.dma_start(out=hsb[:], in_=hs[:])
        nc.gpsimd.collective_compute(
            "AllGather", mybir.AluOpType.bypass,
            replica_groups=[list(range(NCORES))],
            ins=[hsb.opt()], outs=[hs_gat.opt()])

        iota_t = const.tile([P, P], i16)
        nc.gpsimd.iota(iota_t[:], [[1, P]], channel_multiplier=0)
        ident_t = const.tile([P, P], f32)
        make_identity(nc, ident_t[:])

        # W chunk kc spans the gathered W rows of cores 2kc and 2kc+1;
        # dequantize int8 rows with their per-row scales into bf16 w_t.
        wq_full = const.tile([P, KC * D], i8)
        wsc_full = const.tile([P, KC], f32)
        for c in range(NCORES):
            kc, half = divmod(c, 2)
            r0 = c * SROWS + NPC
            nc.sync.dma_start(
                out=wq_full[half * WPC:(half + 1) * WPC, kc * D:(kc + 1) * D],
                in_=hq_gat[r0:r0 + WPC, :])
            nc.sync.dma_start(
                out=wsc_full[half * WPC:(half + 1) * WPC, kc:kc + 1],
                in_=hs_gat[r0:r0 + WPC, :])
        w_t = const.tile([P, KC * D], bf16)
        for kc in range(KC):
            nc.vector.tensor_scalar_mul(
                out=w_t[:, kc * D:(kc + 1) * D],
                in0=wq_full[:, kc * D:(kc + 1) * D],
                scalar1=wsc_full[:, kc:kc + 1])

        for b in range(NBLK):
            e16 = epool.tile([P, 2 * C], i16)
            nc.sync.dma_start(out=e16[:], in_=edg[b])
            idx_b = epool.tile([P, C], i32)
            nc.vector.tensor_copy(out=idx_b[:], in_=e16[:, 0:C])

            # accD[dst, feat] = segment-sum of gathered src rows for this
            # block, accumulated in PSUM across C edge chunks.
            accD = ps1.tile([P, D], f32, space="PSUM")
            for k in range(C):
                gq = gpool.tile([P, D], i8)
                nc.gpsimd.indirect_dma_start(
                    out=gq[:], out_offset=None, in_=hq_gat[:],
                    in_offset=bass.IndirectOffsetOnAxis(ap=idx_b[:, k:k + 1], axis=0),
                )
                gs = gpool.tile([P, 1], f32)
                nc.gpsimd.indirect_dma_start(
                    out=gs[:], out_offset=None, in_=hs_gat[:],
                    in_offset=bass.IndirectOffsetOnAxis(ap=idx_b[:, k:k + 1], axis=0),
                )
                # dequantize: int8 row * per-row scale (exact bf16 scale)
                g_t = gpool.tile([P, D], bf16)
                nc.vector.tensor_scalar_mul(out=g_t[:], in0=gq[:], scalar1=gs[:])
                # S_T[e, j] = (rel[e] == j); padded edges have rel=-1 -> all 0
                s_t = spool.tile([P, P], bf16)
                nc.vector.tensor_tensor(
                    out=s_t[:],
                    in0=e16[:, C + k:C + k + 1].to_broadcast([P, P]),
                    in1=iota_t[:],
                    op=mybir.AluOpType.is_equal,
                )
                nc.tensor.matmul(
                    out=accD[:],
                    lhsT=s_t[:],
                    rhs=g_t[:],
                    start=(k == 0),
                    stop=(k == C - 1),
                )

            accS = apool.tile([P, D], f32)
            nc.vector.tensor_copy(out=accS[:], in_=accD[:])

            # out_ps[dst, :] = sum_kc A_kc @ W_kc (transpose chunks for lhsT)
            out_ps = ps2.tile([P, D], f32, space="PSUM")
            for kc in range(KC):
                tps = pst.tile([P, P], f32, space="PSUM")
                nc.tensor.transpose(
                    out=tps[:], in_=accS[:, kc * P:(kc + 1) * P],
                    identity=ident_t[:])
                lhsT_kc = tpool.tile([P, P], bf16)
                nc.vector.tensor_copy(out=lhsT_kc[:], in_=tps[:])
                nc.tensor.matmul(
                    out=out_ps[:],
                    lhsT=lhsT_kc[:],
                    rhs=w_t[:, kc * D:(kc + 1) * D],
                    start=(kc == 0),
                    stop=(kc == KC - 1),
                )
            # int8-quantize agg@W directly: per-row scale is relative, so
            # the dst-norm multiply and bias add fold exactly into the
            # host-side dequant (out = (q-128)*norm/(127*rinv) + bias).
            out_f = opool.tile([P, D], f32)
            nc.vector.tensor_copy(out=out_f[:], in_=out_ps[:])
            rmax = opool.tile([P, 1], f32)
            nc.vector.tensor_reduce(
                out=rmax[:], in_=out_f[:], axis=mybir.AxisListType.X,
                op=mybir.AluOpType.max, apply_absolute_value=True)
            nc.vector.tensor_scalar_max(out=rmax[:], in0=rmax[:], scalar1=1e-20)
            rinv = opool.tile([P, 1], f32)
            nc.vector.reciprocal(out=rinv[:], in_=rmax[:])
            v_t = opool.tile([P, D], f32)
            nc.vector.tensor_tensor(
                out=v_t[:], in0=out_f[:],
                in1=rinv[:].to_broadcast([P, D]),
                op=mybir.AluOpType.mult,
            )
            # HW f32->u8 cast is round-to-nearest-even with saturation, so a
            # plain +128 offset gives ideal symmetric rounding. (CoreSim
            # truncates instead, inflating sim-reported error only.)
            q_t = opool.tile([P, D], u8)
            nc.vector.tensor_scalar(
                out=q_t[:], in0=v_t[:], scalar1=127.0, scalar2=128.0,
                op0=mybir.AluOpType.mult, op1=mybir.AluOpType.add,
            )
            nc.sync.dma_start(out=out[b * P:(b + 1) * P, :], in_=q_t[:])
            nc.sync.dma_start(out=osc[b * P:(b + 1) * P, :], in_=rinv[:])
    nc.compile()
    return nc


def _prep(h, norm, weight, bias, src, dst):
    # per-core upload rows: 0..NPC-1 node (h*norm) rows, NPC..SROWS-1 W rows;
    # everything int8 with exactly-invertible per-row f32 scales.
    vals = np.zeros((NCORES, SROWS, D), dtype=np.float32)
    hn = h * norm
    for c in range(NCORES):
        lo = c * NPC
        hi = min(N, lo + NPC)
        if lo < N:
            vals[c, :hi - lo, :] = hn[lo:hi]
    vals[:, NPC:, :] = weight.astype(np.float32).reshape(NCORES, WPC, D)

    flat = vals.reshape(NCORES * SROWS, D)
    s = np.abs(flat).max(axis=1, keepdims=True)
    s /= 127.0
    np.maximum(s, 1e-30, out=s)
    np.multiply(flat, 1.0 / s, out=flat)
    np.rint(flat, out=flat)
    q = flat.astype(np.int8).reshape(NCORES, SROWS, D)
    s = s.reshape(NCORES, SROWS, 1)

    src = np.asarray(src, dtype=np.int64)
    dst = np.asarray(dst, dtype=np.int64)
    core_of = dst // NPC
    blk_of = (dst % NPC) // P
    # node n lives at gathered row (n // NPC) * SROWS + (n % NPC)
    src = (src // NPC) * SROWS + (src % NPC)

    # chunk count: max edges landing in any (core, block), ceil to 128
    counts = np.zeros((NCORES, NBLK), dtype=np.int64)
    np.add.at(counts, (core_of, blk_of), 1)
    C = max(1, int(-(-counts.max() // P)))

    edg_all = np.zeros((NCORES, NBLK, P, 2 * C), dtype=np.int16)
    edg_all[:, :, :, C:] = -1
    gkey = core_of * NBLK + blk_of
    order = np.argsort(gkey, kind="stable")
    s_sorted = src[order]
    d_sorted = dst[order]
    g_sorted = gkey[order]
    starts = np.searchsorted(g_sorted, np.arange(NCORES * NBLK))
    rank = np.arange(len(g_sorted)) - starts[g_sorted]
    edg_all[g_sorted // NBLK, g_sorted % NBLK, rank % P, rank // P] = s_sorted
    edg_all[g_sorted // NBLK, g_sorted % NBLK, rank % P, C + rank // P] = (
        d_sorted % P)

    in_maps = []
    for c in range(NCORES):
        in_maps.append({
            "hq": q[c],
            "hs": s[c],
            "edg": edg_all[c],
        })
    return C, in_maps


def _unpack(res, norm, bias):
    """Dequantize (uint8 q, f32 1/rowmax), applying dst-norm and bias."""
    normv = np.zeros((NPAD, 1), dtype=np.float32)
    normv[:N] = norm
    bias = bias.astype(np.float32)[None, :]
    outs = []
    for c in range(NCORES):
        q = np.asarray(res[c]["out"]).astype(np.float32)
        rinv = np.asarray(res[c]["osc"]).astype(np.float32)
        sc = normv[c * NPC:(c + 1) * NPC] / (127.0 * rinv)
        outs.append((q - 128.0) * sc + bias)
    return np.concatenate(outs, axis=0)[:N]


_NC_CACHE = {}


def kernel(h, norm, weight, bias, src, dst):
    h = np.asarray(h, dtype=np.float32)
    norm = np.asarray(norm, dtype=np.float32)
    weight = np.asarray(weight, dtype=np.float32)
    bias = np.asarray(bias, dtype=np.float32)
    C, in_maps = _prep(h, norm, weight, bias, src, dst)
    nc = _NC_CACHE.get(C)
    if nc is None:
        nc = _NC_CACHE[C] = _build(C)
    res = run_bass_kernel_spmd(nc, in_maps, list(range(NCORES))).results
    return _unpack(res, norm, bias)


# revision 26
# speedup vs baseline: 1.1432x; 1.1432x over previous
"""GCN layer on 8 trn2 cores.

Math: out = segment_sum((h@W * norm)[src], dst) * norm + bias
Linearity reorder: out = (segment_sum((h*norm)[src], dst) @ W) * norm + bias
=> aggregate input features first (partitioned by dst), GEMM + epilogue per
   dst shard afterwards.

Host->device traffic is the bottleneck (axon tunnel ~40-60MB/s), so:
- each core uploads only its 1/8 shard of (h*norm) int8-quantized with
  exactly-invertible per-row f32 scales, plus its 1/8 of W in bf16; full
  tables are assembled on-device with AllGathers over NeuronLink
- edge src ids and dst-slot ids go up packed in one int16 tensor
- the output returns int8-quantized (offset-128 uint8) with per-row f32
  reciprocal scales; the dst-norm multiply and bias add fold exactly into
  the host-side dequant
- constants (iota/identity) are generated on-device
- jax persistent compilation cache (keyed per kernel-source hash to avoid
  stale cross-version NEFF collisions) avoids per-process recompiles
"""
import os
import hashlib
import numpy as np
from contextlib import ExitStack

import jax
with open(__file__, "rb") as _f:
    _SRC_HASH = hashlib.sha256(_f.read()).hexdigest()[:16]
jax.config.update("jax_compilation_cache_dir",
                  os.environ.get("KERNEL_JAX_CACHE",
                                 f"/tmp/jax_cache_gcn_{_SRC_HASH}"))
jax.config.update("jax_persistent_cache_min_compile_time_secs", 0)
jax.config.update("jax_persistent_cache_min_entry_size_bytes", 0)

import concourse.bass as bass
import concourse.bacc as bacc
import concourse.mybir as mybir
import concourse.tile as tile
from concourse.masks import make_identity
from concourse.bass_utils import run_bass_kernel_spmd

P = 128
N = 10000
D = 512
NCORES = 8
NPAD = 10240            # N padded to multiple of 128*NCORES
NPC = NPAD // NCORES    # node rows per core = 1280
WPC = D // NCORES       # weight rows per core = 64
SROWS = NPC + WPC       # uploaded rows per core (nodes + W slice) = 1344
GROWS = SROWS * NCORES  # gathered rows = 10752
NBLK = NPC // P         # dst blocks per core = 10
KC = D // P             # feature chunks = 4


def _build(C):
    """Build the single SPMD Bass program. C = edge chunks per dst block."""
    nc = bacc.Bacc(None, target_bir_lowering=False)
    f32 = mybir.dt.float32
    bf16 = mybir.dt.bfloat16
    i32 = mybir.dt.int32
    i16 = mybir.dt.int16
    i8 = mybir.dt.int8
    u8 = mybir.dt.uint8

    hq = nc.declare_dram_parameter("hq", [SROWS, D], i8, isOutput=False)
    hs = nc.declare_dram_parameter("hs", [SROWS, 1], f32, isOutput=False)
    edg = nc.declare_dram_parameter("edg", [NBLK, P, 3 * C], u8, isOutput=False)
    out = nc.declare_dram_parameter("out", [NPC, D], u8, isOutput=True)
    osc = nc.declare_dram_parameter("osc", [NPC, 1], f32, isOutput=True)

    with tile.TileContext(nc) as tc, ExitStack() as ctx:
        dram = ctx.enter_context(tc.tile_pool(name="dram", bufs=6, space="DRAM"))
        const = ctx.enter_context(tc.tile_pool(name="const", bufs=1))
        epool = ctx.enter_context(tc.tile_pool(name="edges", bufs=NBLK))
        gpool = ctx.enter_context(tc.tile_pool(name="gath", bufs=8))
        spool = ctx.enter_context(tc.tile_pool(name="sel", bufs=8))
        apool = ctx.enter_context(tc.tile_pool(name="accs", bufs=NBLK))
        tpool = ctx.enter_context(tc.tile_pool(name="trs", bufs=4 * NBLK))
        opool = ctx.enter_context(tc.tile_pool(name="outs", bufs=2 * NBLK))
        ps1 = ctx.enter_context(tc.tile_pool(name="ps1", bufs=2, space="PSUM"))
        pst = ctx.enter_context(tc.tile_pool(name="pst", bufs=4, space="PSUM"))
        ps2 = ctx.enter_context(tc.tile_pool(name="ps2", bufs=2, space="PSUM"))

        # Assemble full tables on-device: each core uploads its 1/8 of the
        # int8 node rows plus its 1/8 of int8 W rows (and their f32 scales);
        # AllGather moves the rest over NeuronLink.
        hqb = dram.tile([SROWS, D], i8)
        hq_gat = dram.tile([GROWS, D], i8)
        nc.gpsimd.dma_start(out=hqb[:], in_=hq[:])
        nc.gpsimd.collective_compute(
            "AllGather", mybir.AluOpType.bypass,
            replica_groups=[list(range(NCORES))],
            ins=[hqb.opt()], outs=[hq_gat.opt()])

        hsb = dram.tile([SROWS, 1], f32)
        hs_gat = dram.tile([GROWS, 1], f32)
        nc.gpsimd.dma_start(out=hsb[:], in_=hs[:])
        nc.gpsimd.collective_compute(
            "AllGather", mybir.AluOpType.bypass,
            replica_groups=[list(range(NCORES))],
            ins=[hsb.opt()], outs=[hs_gat.opt()])

        iota_i16 = const.tile([P, P], i16)
        nc.gpsimd.iota(iota_i16[:], [[1, P]], channel_multiplier=0)
        iota_t = const.tile([P, P], u8)
        nc.vector.tensor_copy(out=iota_t[:], in_=iota_i16[:])
        ident_t = const.tile([P, P], f32)
        make_identity(nc, ident_t[:])

        # W chunk kc spans the gathered W rows of cores 2kc and 2kc+1;
        # dequantize int8 rows with their per-row scales into bf16 w_t.
        wq_full = const.tile([P, KC * D], i8)
        wsc_full = const.tile([P, KC], f32)
        for c in range(NCORES):
            kc, half = divmod(c, 2)
            r0 = c * SROWS + NPC
            nc.sync.dma_start(
                out=wq_full[half * WPC:(half + 1) * WPC, kc * D:(kc + 1) * D],
                in_=hq_gat[r0:r0 + WPC, :])
            nc.sync.dma_start(
                out=wsc_full[half * WPC:(half + 1) * WPC, kc:kc + 1],
                in_=hs_gat[r0:r0 + WPC, :])
        w_t = const.tile([P, KC * D], bf16)
        for kc in range(KC):
            nc.vector.tensor_scalar_mul(
                out=w_t[:, kc * D:(kc + 1) * D],
                in0=wq_full[:, kc * D:(kc + 1) * D],
                scalar1=wsc_full[:, kc:kc + 1])

        for b in range(NBLK):
            # edge slots: cols 0..C-1 idx lo byte, C..2C-1 idx hi byte,
            # 2C..3C-1 dst-slot (255 = padding). gpsimd DMAs value-cast u8->i32.
            e8 = epool.tile([P, 3 * C], u8)
            nc.sync.dma_start(out=e8[:], in_=edg[b])
            lo32 = epool.tile([P, C], i32)
            nc.gpsimd.dma_start(out=lo32[:], in_=edg[b][:, 0:C])
            hi32 = epool.tile([P, C], i32)
            nc.gpsimd.dma_start(out=hi32[:], in_=edg[b][:, C:2 * C])
            idx_b = epool.tile([P, C], i32)
            nc.vector.tensor_scalar(
                out=idx_b[:], in0=hi32[:], scalar1=256, scalar2=None,
                op0=mybir.AluOpType.mult)
            nc.vector.tensor_tensor(
                out=idx_b[:], in0=idx_b[:], in1=lo32[:],
                op=mybir.AluOpType.add)

            # accD[dst, feat] = segment-sum of gathered src rows for this
            # block, accumulated in PSUM across C edge chunks.
            accD = ps1.tile([P, D], f32, space="PSUM")
            for k in range(C):
                gq = gpool.tile([P, D], i8)
                nc.gpsimd.indirect_dma_start(
                    out=gq[:], out_offset=None, in_=hq_gat[:],
                    in_offset=bass.IndirectOffsetOnAxis(ap=idx_b[:, k:k + 1], axis=0),
                )
                gs = gpool.tile([P, 1], f32)
                nc.gpsimd.indirect_dma_start(
                    out=gs[:], out_offset=None, in_=hs_gat[:],
                    in_offset=bass.IndirectOffsetOnAxis(ap=idx_b[:, k:k + 1], axis=0),
                )
                # dequantize: int8 row * per-row scale (exact bf16 scale)
                g_t = gpool.tile([P, D], bf16)
                nc.vector.tensor_scalar_mul(out=g_t[:], in0=gq[:], scalar1=gs[:])
                # S_T[e, j] = (rel[e] == j); padded edges have rel=-1 -> all 0
                s_t = spool.tile([P, P], bf16)
                nc.vector.tensor_tensor(
                    out=s_t[:],
                    in0=e8[:, 2 * C + k:2 * C + k + 1].to_broadcast([P, P]),
                    in1=iota_t[:],
                    op=mybir.AluOpType.is_equal,
                )
                nc.tensor.matmul(
                    out=accD[:],
                    lhsT=s_t[:],
                    rhs=g_t[:],
                    start=(k == 0),
                    stop=(k == C - 1),
                )

            accS = apool.tile([P, D], f32)
            nc.vector.tensor_copy(out=accS[:], in_=accD[:])

            # out_ps[dst, :] = sum_kc A_kc @ W_kc (transpose chunks for lhsT)
            out_ps = ps2.tile([P, D], f32, space="PSUM")
            for kc in range(KC):
                tps = pst.tile([P, P], f32, space="PSUM")
                nc.tensor.transpose(
                    out=tps[:], in_=accS[:, kc * P:(kc + 1) * P],
                    identity=ident_t[:])
                lhsT_kc = tpool.tile([P, P], bf16)
                nc.vector.tensor_copy(out=lhsT_kc[:], in_=tps[:])
                nc.tensor.matmul(
                    out=out_ps[:],
                    lhsT=lhsT_kc[:],
                    rhs=w_t[:, kc * D:(kc + 1) * D],
                    start=(kc == 0),
                    stop=(kc == KC - 1),
                )
            # int8-quantize agg@W directly: per-row scale is relative, so
            # the dst-norm multiply and bias add fold exactly into the
            # host-side dequant (out = (q-128)*norm/(127*rinv) + bias).
            out_f = opool.tile([P, D], f32)
            nc.vector.tensor_copy(out=out_f[:], in_=out_ps[:])
            rmax = opool.tile([P, 1], f32)
            nc.vector.tensor_reduce(
                out=rmax[:], in_=out_f[:], axis=mybir.AxisListType.X,
                op=mybir.AluOpType.max, apply_absolute_value=True)
            nc.vector.tensor_scalar_max(out=rmax[:], in0=rmax[:], scalar1=1e-20)
            rinv = opool.tile([P, 1], f32)
            nc.vector.reciprocal(out=rinv[:], in_=rmax[:])
            v_t = opool.tile([P, D], f32)
            nc.vector.tensor_tensor(
                out=v_t[:], in0=out_f[:],
                in1=rinv[:].to_broadcast([P, D]),
                op=mybir.AluOpType.mult,
            )
            # HW f32->u8 cast is round-to-nearest-even with saturation, so a
            # plain +128 offset gives ideal symmetric rounding. (CoreSim
            # truncates instead, inflating sim-reported error only.)
            q_t = opool.tile([P, D], u8)
            nc.vector.tensor_scalar(
                out=q_t[:], in0=v_t[:], scalar1=127.0, scalar2=128.0,
                op0=mybir.AluOpType.mult, op1=mybir.AluOpType.add,
            )
            nc.sync.dma_start(out=out[b * P:(b + 1) * P, :], in_=q_t[:])
            nc.sync.dma_start(out=osc[b * P:(b + 1) * P, :], in_=rinv[:])
    nc.compile()
    return nc


def _prep(h, norm, weight, bias, src, dst):
    # per-core upload rows: 0..NPC-1 node (h*norm) rows, NPC..SROWS-1 W rows;
    # everything int8 with exactly-invertible per-row f32 scales.
    vals = np.zeros((NCORES, SROWS, D), dtype=np.float32)
    hn = h * norm
    for c in range(NCORES):
        lo = c * NPC
        hi = min(N, lo + NPC)
        if lo < N:
            vals[c, :hi - lo, :] = hn[lo:hi]
    vals[:, NPC:, :] = weight.astype(np.float32).reshape(NCORES, WPC, D)

    flat = vals.reshape(NCORES * SROWS, D)
    s = np.abs(flat).max(axis=1, keepdims=True)
    s /= 127.0
    np.maximum(s, 1e-30, out=s)
    np.multiply(flat, 1.0 / s, out=flat)
    np.rint(flat, out=flat)
    q = flat.astype(np.int8).reshape(NCORES, SROWS, D)
    s = s.reshape(NCORES, SROWS, 1)

    src = np.asarray(src, dtype=np.int64)
    dst = np.asarray(dst, dtype=np.int64)
    core_of = dst // NPC
    blk_of = (dst % NPC) // P
    # node n lives at gathered row (n // NPC) * SROWS + (n % NPC)
    src = (src // NPC) * SROWS + (src % NPC)

    # chunk count: max edges landing in any (core, block), ceil to 128
    counts = np.zeros((NCORES, NBLK), dtype=np.int64)
    np.add.at(counts, (core_of, blk_of), 1)
    C = max(1, int(-(-counts.max() // P)))

    edg_all = np.zeros((NCORES, NBLK, P, 3 * C), dtype=np.uint8)
    edg_all[:, :, :, 2 * C:] = 255
    gkey = core_of * NBLK + blk_of
    order = np.argsort(gkey, kind="stable")
    s_sorted = src[order]
    d_sorted = dst[order]
    g_sorted = gkey[order]
    starts = np.searchsorted(g_sorted, np.arange(NCORES * NBLK))
    rank = np.arange(len(g_sorted)) - starts[g_sorted]
    cc, bb, pp, kk = (g_sorted // NBLK, g_sorted % NBLK, rank % P, rank // P)
    edg_all[cc, bb, pp, kk] = s_sorted & 0xFF
    edg_all[cc, bb, pp, C + kk] = s_sorted >> 8
    edg_all[cc, bb, pp, 2 * C + kk] = d_sorted % P

    in_maps = []
    for c in range(NCORES):
        in_maps.append({
            "hq": q[c],
            "hs": s[c],
            "edg": edg_all[c],
        })
    return C, in_maps


def _unpack(res, norm, bias):
    """Dequantize (uint8 q, f32 1/rowmax), applying dst-norm and bias."""
    normv = np.zeros((NPAD, 1), dtype=np.float32)
    normv[:N] = norm
    bias = bias.astype(np.float32)[None, :]
    outs = []
    for c in range(NCORES):
        q = np.asarray(res[c]["out"]).astype(np.float32)
        rinv = np.asarray(res[c]["osc"]).astype(np.float32)
        sc = normv[c * NPC:(c + 1) * NPC] / (127.0 * rinv)
        outs.append((q - 128.0) * sc + bias)
    return np.concatenate(outs, axis=0)[:N]


_NC_CACHE = {}


def kernel(h, norm, weight, bias, src, dst):
    h = np.asarray(h, dtype=np.float32)
    norm = np.asarray(norm, dtype=np.float32)
    weight = np.asarray(weight, dtype=np.float32)
    bias = np.asarray(bias, dtype=np.float32)
    C, in_maps = _prep(h, norm, weight, bias, src, dst)
    nc = _NC_CACHE.get(C)
    if nc is None:
        nc = _NC_CACHE[C] = _build(C)
    res = run_bass_kernel_spmd(nc, in_maps, list(range(NCORES))).results
    return _unpack(res, norm, bias)


# revision 29
# speedup vs baseline: 1.3292x; 1.1626x over previous
"""GCN layer on 8 trn2 cores.

Math: out = segment_sum((h@W * norm)[src], dst) * norm + bias
Linearity reorder: out = (segment_sum((h*norm)[src], dst) @ W) * norm + bias
=> aggregate input features first (partitioned by dst), GEMM + epilogue per
   dst shard afterwards.

Host->device traffic is the bottleneck (axon tunnel ~40-60MB/s), so:
- each core uploads only its 1/8 shard of (h*norm) int8-quantized with
  exactly-invertible per-row f32 scales, plus its 1/8 of W in bf16; full
  tables are assembled on-device with AllGathers over NeuronLink
- edge src ids and dst-slot ids go up packed in one int16 tensor
- the output returns int8-quantized (offset-128 uint8) with per-row f32
  reciprocal scales; the dst-norm multiply and bias add fold exactly into
  the host-side dequant
- constants (iota/identity) are generated on-device
- jax persistent compilation cache (keyed per kernel-source hash to avoid
  stale cross-version NEFF collisions) avoids per-process recompiles
"""
import os
import hashlib
import numpy as np
from contextlib import ExitStack

import jax
with open(__file__, "rb") as _f:
    _SRC_HASH = hashlib.sha256(_f.read()).hexdigest()[:16]
jax.config.update("jax_compilation_cache_dir",
                  os.environ.get("KERNEL_JAX_CACHE",
                                 f"/tmp/jax_cache_gcn_{_SRC_HASH}"))
jax.config.update("jax_persistent_cache_min_compile_time_secs", 0)
jax.config.update("jax_persistent_cache_min_entry_size_bytes", 0)

import concourse.bass as bass
import concourse.bacc as bacc
import concourse.mybir as mybir
import concourse.tile as tile
from concourse.masks import make_identity
from concourse.bass_utils import run_bass_kernel_spmd

P = 128
N = 10000
D = 512
NCORES = 8
NPAD = 10240            # N padded to multiple of 128*NCORES
NPC = NPAD // NCORES    # node rows per core = 1280
WPC = D // NCORES       # weight rows per core = 64
SROWS = NPC + WPC       # uploaded rows per core (nodes + W slice) = 1344
GROWS = SROWS * NCORES  # gathered rows = 10752
NBLK = NPC // P         # dst blocks per core = 10
KC = D // P             # feature chunks = 4


def _build(C):
    """Build the single SPMD Bass program. C = edge chunks per dst block."""
    nc = bacc.Bacc(None, target_bir_lowering=False)
    f32 = mybir.dt.float32
    bf16 = mybir.dt.bfloat16
    i32 = mybir.dt.int32
    i16 = mybir.dt.int16
    i8 = mybir.dt.int8
    u8 = mybir.dt.uint8

    hq = nc.declare_dram_parameter("hq", [SROWS, D], i8, isOutput=False)
    hs = nc.declare_dram_parameter("hs", [SROWS, 1], f32, isOutput=False)
    edg = nc.declare_dram_parameter("edg", [NBLK, P, 3 * C], u8, isOutput=False)
    out = nc.declare_dram_parameter("out", [NPC, D + 1], u8, isOutput=True)

    with tile.TileContext(nc) as tc, ExitStack() as ctx:
        dram = ctx.enter_context(tc.tile_pool(name="dram", bufs=6, space="DRAM"))
        const = ctx.enter_context(tc.tile_pool(name="const", bufs=1))
        epool = ctx.enter_context(tc.tile_pool(name="edges", bufs=NBLK))
        gpool = ctx.enter_context(tc.tile_pool(name="gath", bufs=8))
        spool = ctx.enter_context(tc.tile_pool(name="sel", bufs=8))
        apool = ctx.enter_context(tc.tile_pool(name="accs", bufs=NBLK))
        tpool = ctx.enter_context(tc.tile_pool(name="trs", bufs=4 * NBLK))
        opool = ctx.enter_context(tc.tile_pool(name="outs", bufs=2 * NBLK))
        ps1 = ctx.enter_context(tc.tile_pool(name="ps1", bufs=2, space="PSUM"))
        pst = ctx.enter_context(tc.tile_pool(name="pst", bufs=4, space="PSUM"))
        ps2 = ctx.enter_context(tc.tile_pool(name="ps2", bufs=2, space="PSUM"))

        # Assemble full tables on-device: each core uploads its 1/8 of the
        # int8 node rows plus its 1/8 of int8 W rows (and their f32 scales);
        # AllGather moves the rest over NeuronLink.
        hqb = dram.tile([SROWS, D], i8)
        hq_gat = dram.tile([GROWS, D], i8)
        nc.gpsimd.dma_start(out=hqb[:], in_=hq[:])
        nc.gpsimd.collective_compute(
            "AllGather", mybir.AluOpType.bypass,
            replica_groups=[list(range(NCORES))],
            ins=[hqb.opt()], outs=[hq_gat.opt()])

        hsb = dram.tile([SROWS, 1], f32)
        hs_gat = dram.tile([GROWS, 1], f32)
        nc.gpsimd.dma_start(out=hsb[:], in_=hs[:])
        nc.gpsimd.collective_compute(
            "AllGather", mybir.AluOpType.bypass,
            replica_groups=[list(range(NCORES))],
            ins=[hsb.opt()], outs=[hs_gat.opt()])

        iota_i16 = const.tile([P, P], i16)
        nc.gpsimd.iota(iota_i16[:], [[1, P]], channel_multiplier=0)
        iota_t = const.tile([P, P], u8)
        nc.vector.tensor_copy(out=iota_t[:], in_=iota_i16[:])
        ident_t = const.tile([P, P], f32)
        make_identity(nc, ident_t[:])

        # W chunk kc spans the gathered W rows of cores 2kc and 2kc+1;
        # dequantize int8 rows with their per-row scales into bf16 w_t.
        wq_full = const.tile([P, KC * D], i8)
        wsc_full = const.tile([P, KC], f32)
        for c in range(NCORES):
            kc, half = divmod(c, 2)
            r0 = c * SROWS + NPC
            nc.sync.dma_start(
                out=wq_full[half * WPC:(half + 1) * WPC, kc * D:(kc + 1) * D],
                in_=hq_gat[r0:r0 + WPC, :])
            nc.sync.dma_start(
                out=wsc_full[half * WPC:(half + 1) * WPC, kc:kc + 1],
                in_=hs_gat[r0:r0 + WPC, :])
        w_t = const.tile([P, KC * D], bf16)
        for kc in range(KC):
            nc.vector.tensor_scalar_mul(
                out=w_t[:, kc * D:(kc + 1) * D],
                in0=wq_full[:, kc * D:(kc + 1) * D],
                scalar1=wsc_full[:, kc:kc + 1])

        for b in range(NBLK):
            # edge slots: cols 0..C-1 idx lo byte, C..2C-1 idx hi byte,
            # 2C..3C-1 dst-slot (255 = padding). gpsimd DMAs value-cast u8->i32.
            e8 = epool.tile([P, 3 * C], u8)
            nc.sync.dma_start(out=e8[:], in_=edg[b])
            lo32 = epool.tile([P, C], i32)
            nc.gpsimd.dma_start(out=lo32[:], in_=edg[b][:, 0:C])
            hi32 = epool.tile([P, C], i32)
            nc.gpsimd.dma_start(out=hi32[:], in_=edg[b][:, C:2 * C])
            idx_b = epool.tile([P, C], i32)
            nc.vector.tensor_scalar(
                out=idx_b[:], in0=hi32[:], scalar1=256, scalar2=None,
                op0=mybir.AluOpType.mult)
            nc.vector.tensor_tensor(
                out=idx_b[:], in0=idx_b[:], in1=lo32[:],
                op=mybir.AluOpType.add)

            # accD[dst, feat] = segment-sum of gathered src rows for this
            # block, accumulated in PSUM across C edge chunks.
            accD = ps1.tile([P, D], f32, space="PSUM")
            for k in range(C):
                gq = gpool.tile([P, D], i8)
                nc.gpsimd.indirect_dma_start(
                    out=gq[:], out_offset=None, in_=hq_gat[:],
                    in_offset=bass.IndirectOffsetOnAxis(ap=idx_b[:, k:k + 1], axis=0),
                )
                gs = gpool.tile([P, 1], f32)
                nc.gpsimd.indirect_dma_start(
                    out=gs[:], out_offset=None, in_=hs_gat[:],
                    in_offset=bass.IndirectOffsetOnAxis(ap=idx_b[:, k:k + 1], axis=0),
                )
                # dequantize: int8 row * per-row scale (exact bf16 scale)
                g_t = gpool.tile([P, D], bf16)
                nc.vector.tensor_scalar_mul(out=g_t[:], in0=gq[:], scalar1=gs[:])
                # S_T[e, j] = (rel[e] == j); padded edges have rel=-1 -> all 0
                s_t = spool.tile([P, P], bf16)
                nc.vector.tensor_tensor(
                    out=s_t[:],
                    in0=e8[:, 2 * C + k:2 * C + k + 1].to_broadcast([P, P]),
                    in1=iota_t[:],
                    op=mybir.AluOpType.is_equal,
                )
                nc.tensor.matmul(
                    out=accD[:],
                    lhsT=s_t[:],
                    rhs=g_t[:],
                    start=(k == 0),
                    stop=(k == C - 1),
                )

            accS = apool.tile([P, D], f32)
            nc.vector.tensor_copy(out=accS[:], in_=accD[:])

            # out_ps[dst, :] = sum_kc A_kc @ W_kc (transpose chunks for lhsT)
            out_ps = ps2.tile([P, D], f32, space="PSUM")
            for kc in range(KC):
                tps = pst.tile([P, P], f32, space="PSUM")
                nc.tensor.transpose(
                    out=tps[:], in_=accS[:, kc * P:(kc + 1) * P],
                    identity=ident_t[:])
                lhsT_kc = tpool.tile([P, P], bf16)
                nc.vector.tensor_copy(out=lhsT_kc[:], in_=tps[:])
                nc.tensor.matmul(
                    out=out_ps[:],
                    lhsT=lhsT_kc[:],
                    rhs=w_t[:, kc * D:(kc + 1) * D],
                    start=(kc == 0),
                    stop=(kc == KC - 1),
                )
            # int8-quantize agg@W directly: per-row scale is relative, so
            # the dst-norm multiply and bias add fold exactly into the
            # host-side dequant. The scale ships as one u8 exponent byte
            # e = RN(16*log2(rmax)+129.5) (so decoded s' >= rmax); device
            # and host both decode s' = 2^((e-128)/16), keeping dequant
            # consistent up to the Exp LUT's tiny approximation error.
            out_f = opool.tile([P, D], f32)
            nc.vector.tensor_copy(out=out_f[:], in_=out_ps[:])
            rmax = opool.tile([P, 1], f32)
            nc.vector.tensor_reduce(
                out=rmax[:], in_=out_f[:], axis=mybir.AxisListType.X,
                op=mybir.AluOpType.max, apply_absolute_value=True)
            nc.vector.tensor_scalar_max(out=rmax[:], in0=rmax[:], scalar1=1e-20)
            kf = opool.tile([P, 1], f32)
            nc.scalar.activation(out=kf[:], in_=rmax[:],
                                 func=mybir.ActivationFunctionType.Ln)
            e8 = opool.tile([P, 1], u8)
            nc.vector.tensor_scalar(
                out=e8[:], in0=kf[:], scalar1=23.083120654223414,
                scalar2=129.5, op0=mybir.AluOpType.mult,
                op1=mybir.AluOpType.add)
            ef = opool.tile([P, 1], f32)
            nc.vector.tensor_copy(out=ef[:], in_=e8[:])
            nc.vector.tensor_scalar_add(out=ef[:], in0=ef[:], scalar1=-128.0)
            rinv = opool.tile([P, 1], f32)
            nc.scalar.activation(out=rinv[:], in_=ef[:],
                                 func=mybir.ActivationFunctionType.Exp,
                                 scale=-0.04332169878499658)
            v_t = opool.tile([P, D], f32)
            nc.vector.tensor_tensor(
                out=v_t[:], in0=out_f[:],
                in1=rinv[:].to_broadcast([P, D]),
                op=mybir.AluOpType.mult,
            )
            # HW f32->u8 cast is round-to-nearest-even with saturation, so a
            # plain +128 offset gives ideal symmetric rounding. (CoreSim
            # truncates instead, inflating sim-reported error only.)
            q_t = opool.tile([P, D], u8)
            nc.vector.tensor_scalar(
                out=q_t[:], in0=v_t[:], scalar1=127.0, scalar2=128.0,
                op0=mybir.AluOpType.mult, op1=mybir.AluOpType.add,
            )
            nc.sync.dma_start(out=out[b * P:(b + 1) * P, 0:D], in_=q_t[:])
            nc.sync.dma_start(out=out[b * P:(b + 1) * P, D:D + 1], in_=e8[:])
    nc.compile()
    return nc


def _prep(h, norm, weight, bias, src, dst):
    # per-core upload rows: 0..NPC-1 node (h*norm) rows, NPC..SROWS-1 W rows;
    # everything int8 with exactly-invertible per-row f32 scales.
    vals = np.zeros((NCORES, SROWS, D), dtype=np.float32)
    hn = h * norm
    for c in range(NCORES):
        lo = c * NPC
        hi = min(N, lo + NPC)
        if lo < N:
            vals[c, :hi - lo, :] = hn[lo:hi]
    vals[:, NPC:, :] = weight.astype(np.float32).reshape(NCORES, WPC, D)

    flat = vals.reshape(NCORES * SROWS, D)
    s = np.abs(flat).max(axis=1, keepdims=True)
    s /= 127.0
    np.maximum(s, 1e-30, out=s)
    np.multiply(flat, 1.0 / s, out=flat)
    np.rint(flat, out=flat)
    q = flat.astype(np.int8).reshape(NCORES, SROWS, D)
    s = s.reshape(NCORES, SROWS, 1)

    src = np.asarray(src, dtype=np.int64)
    dst = np.asarray(dst, dtype=np.int64)
    core_of = dst // NPC
    blk_of = (dst % NPC) // P
    # node n lives at gathered row (n // NPC) * SROWS + (n % NPC)
    src = (src // NPC) * SROWS + (src % NPC)

    # chunk count: max edges landing in any (core, block), ceil to 128
    counts = np.zeros((NCORES, NBLK), dtype=np.int64)
    np.add.at(counts, (core_of, blk_of), 1)
    C = max(1, int(-(-counts.max() // P)))

    edg_all = np.zeros((NCORES, NBLK, P, 3 * C), dtype=np.uint8)
    edg_all[:, :, :, 2 * C:] = 255
    gkey = core_of * NBLK + blk_of
    order = np.argsort(gkey, kind="stable")
    s_sorted = src[order]
    d_sorted = dst[order]
    g_sorted = gkey[order]
    starts = np.searchsorted(g_sorted, np.arange(NCORES * NBLK))
    rank = np.arange(len(g_sorted)) - starts[g_sorted]
    cc, bb, pp, kk = (g_sorted // NBLK, g_sorted % NBLK, rank % P, rank // P)
    edg_all[cc, bb, pp, kk] = s_sorted & 0xFF
    edg_all[cc, bb, pp, C + kk] = s_sorted >> 8
    edg_all[cc, bb, pp, 2 * C + kk] = d_sorted % P

    in_maps = []
    for c in range(NCORES):
        in_maps.append({
            "hq": q[c],
            "hs": s[c],
            "edg": edg_all[c],
        })
    return C, in_maps


def _unpack(res, norm, bias):
    """Dequantize (uint8 q | u8 scale-exponent) rows, applying dst-norm and
    bias: out = (q-128) * 2^((e-128)/16) / 127 * norm + bias."""
    normv = np.zeros((NPAD, 1), dtype=np.float32)
    normv[:N] = norm
    bias = bias.astype(np.float32)[None, :]
    outs = []
    for c in range(NCORES):
        raw = np.asarray(res[c]["out"])
        q = raw[:, :D].astype(np.float32)
        e = raw[:, D:D + 1].astype(np.float32)
        s = np.exp2((e - 128.0) / 16.0).astype(np.float32)
        sc = normv[c * NPC:(c + 1) * NPC] * (s / 127.0)
        outs.append((q - 128.0) * sc + bias)
    return np.concatenate(outs, axis=0)[:N]


_NC_CACHE = {}


def kernel(h, norm, weight, bias, src, dst):
    h = np.asarray(h, dtype=np.float32)
    norm = np.asarray(norm, dtype=np.float32)
    weight = np.asarray(weight, dtype=np.float32)
    bias = np.asarray(bias, dtype=np.float32)
    C, in_maps = _prep(h, norm, weight, bias, src, dst)
    nc = _NC_CACHE.get(C)
    if nc is None:
        nc = _NC_CACHE[C] = _build(C)
    res = run_bass_kernel_spmd(nc, in_maps, list(range(NCORES))).results
    return _unpack(res, norm, bias)


# revision 30
# speedup vs baseline: 1.3496x; 1.0154x over previous
"""GCN layer on 8 trn2 cores.

Math: out = segment_sum((h@W * norm)[src], dst) * norm + bias
Linearity reorder: out = (segment_sum((h*norm)[src], dst) @ W) * norm + bias
=> aggregate input features first (partitioned by dst), GEMM + epilogue per
   dst shard afterwards.

Host->device traffic is the bottleneck (axon tunnel ~40-60MB/s), so:
- each core uploads only its 1/8 shard of (h*norm) int8-quantized with
  exactly-invertible per-row f32 scales, plus its 1/8 of W in bf16; full
  tables are assembled on-device with AllGathers over NeuronLink
- edge src ids and dst-slot ids go up packed in one int16 tensor
- the output returns int8-quantized (offset-128 uint8) with per-row f32
  reciprocal scales; the dst-norm multiply and bias add fold exactly into
  the host-side dequant
- constants (iota/identity) are generated on-device
- jax persistent compilation cache (keyed per kernel-source hash to avoid
  stale cross-version NEFF collisions) avoids per-process recompiles
"""
import os
import hashlib
import numpy as np
from contextlib import ExitStack

import jax
with open(__file__, "rb") as _f:
    _SRC_HASH = hashlib.sha256(_f.read()).hexdigest()[:16]
jax.config.update("jax_compilation_cache_dir",
                  os.environ.get("KERNEL_JAX_CACHE",
                                 f"/tmp/jax_cache_gcn_{_SRC_HASH}"))
jax.config.update("jax_persistent_cache_min_compile_time_secs", 0)
jax.config.update("jax_persistent_cache_min_entry_size_bytes", 0)

import concourse.bass as bass
import concourse.bacc as bacc
import concourse.mybir as mybir
import concourse.tile as tile
from concourse.masks import make_identity
from concourse.bass_utils import run_bass_kernel_spmd

P = 128
N = 10000
D = 512
NCORES = 8
NPAD = 10240            # N padded to multiple of 128*NCORES
NPC = NPAD // NCORES    # node rows per core = 1280
WPC = D // NCORES       # weight rows per core = 64
SROWS = NPC + WPC       # uploaded rows per core (nodes + W slice) = 1344
GROWS = SROWS * NCORES  # gathered rows = 10752
NBLK = NPC // P         # dst blocks per core = 10
KC = D // P             # feature chunks = 4


def _build(C):
    """Build the single SPMD Bass program. C = edge chunks per dst block."""
    nc = bacc.Bacc(None, target_bir_lowering=False)
    f32 = mybir.dt.float32
    bf16 = mybir.dt.bfloat16
    i32 = mybir.dt.int32
    i16 = mybir.dt.int16
    i8 = mybir.dt.int8
    u8 = mybir.dt.uint8

    hq = nc.declare_dram_parameter("hq", [SROWS, D + 1], i8, isOutput=False)
    edg = nc.declare_dram_parameter("edg", [NBLK, P, 3 * C], u8, isOutput=False)
    out = nc.declare_dram_parameter("out", [NPC, D + 1], u8, isOutput=True)

    with tile.TileContext(nc) as tc, ExitStack() as ctx:
        dram = ctx.enter_context(tc.tile_pool(name="dram", bufs=6, space="DRAM"))
        const = ctx.enter_context(tc.tile_pool(name="const", bufs=1))
        epool = ctx.enter_context(tc.tile_pool(name="edges", bufs=NBLK))
        gpool = ctx.enter_context(tc.tile_pool(name="gath", bufs=8))
        spool = ctx.enter_context(tc.tile_pool(name="sel", bufs=8))
        apool = ctx.enter_context(tc.tile_pool(name="accs", bufs=NBLK))
        tpool = ctx.enter_context(tc.tile_pool(name="trs", bufs=4 * NBLK))
        opool = ctx.enter_context(tc.tile_pool(name="outs", bufs=2 * NBLK))
        ps1 = ctx.enter_context(tc.tile_pool(name="ps1", bufs=2, space="PSUM"))
        pst = ctx.enter_context(tc.tile_pool(name="pst", bufs=4, space="PSUM"))
        ps2 = ctx.enter_context(tc.tile_pool(name="ps2", bufs=2, space="PSUM"))

        # Assemble full tables on-device: each core uploads its 1/8 of the
        # int8 node rows plus its 1/8 of int8 W rows (and their f32 scales);
        # AllGather moves the rest over NeuronLink.
        hqb = dram.tile([SROWS, D + 1], i8)
        hq_gat = dram.tile([GROWS, D + 1], i8)
        nc.gpsimd.dma_start(out=hqb[:], in_=hq[:])
        nc.gpsimd.collective_compute(
            "AllGather", mybir.AluOpType.bypass,
            replica_groups=[list(range(NCORES))],
            ins=[hqb.opt()], outs=[hq_gat.opt()])

        iota_i16 = const.tile([P, P], i16)
        nc.gpsimd.iota(iota_i16[:], [[1, P]], channel_multiplier=0)
        iota_t = const.tile([P, P], u8)
        nc.vector.tensor_copy(out=iota_t[:], in_=iota_i16[:])
        ident_t = const.tile([P, P], f32)
        make_identity(nc, ident_t[:])

        # W chunk kc spans the gathered W rows of cores 2kc and 2kc+1;
        # dequantize int8 rows with their per-row scale-exponent bytes
        # (col D): s = 2^((e8 - 192)/16), decoded via the Exp activation.
        wq_full = const.tile([P, KC * D], i8)
        ws8_full = const.tile([P, KC], i8)
        for c in range(NCORES):
            kc, half = divmod(c, 2)
            r0 = c * SROWS + NPC
            nc.sync.dma_start(
                out=wq_full[half * WPC:(half + 1) * WPC, kc * D:(kc + 1) * D],
                in_=hq_gat[r0:r0 + WPC, 0:D])
            nc.sync.dma_start(
                out=ws8_full[half * WPC:(half + 1) * WPC, kc:kc + 1],
                in_=hq_gat[r0:r0 + WPC, D:D + 1])
        wsc_full = const.tile([P, KC], f32)
        nc.vector.tensor_copy(out=wsc_full[:], in_=ws8_full[:])
        nc.vector.tensor_scalar_add(out=wsc_full[:], in0=wsc_full[:],
                                    scalar1=-192.0)
        nc.scalar.activation(out=wsc_full[:], in_=wsc_full[:],
                             func=mybir.ActivationFunctionType.Exp,
                             scale=0.04332169878499658)
        w_t = const.tile([P, KC * D], bf16)
        for kc in range(KC):
            nc.vector.tensor_scalar_mul(
                out=w_t[:, kc * D:(kc + 1) * D],
                in0=wq_full[:, kc * D:(kc + 1) * D],
                scalar1=wsc_full[:, kc:kc + 1])

        for b in range(NBLK):
            # edge slots: cols 0..C-1 idx lo byte, C..2C-1 idx hi byte,
            # 2C..3C-1 dst-slot (255 = padding). gpsimd DMAs value-cast u8->i32.
            e8 = epool.tile([P, 3 * C], u8)
            nc.sync.dma_start(out=e8[:], in_=edg[b])
            lo32 = epool.tile([P, C], i32)
            nc.gpsimd.dma_start(out=lo32[:], in_=edg[b][:, 0:C])
            hi32 = epool.tile([P, C], i32)
            nc.gpsimd.dma_start(out=hi32[:], in_=edg[b][:, C:2 * C])
            idx_b = epool.tile([P, C], i32)
            nc.vector.tensor_scalar(
                out=idx_b[:], in0=hi32[:], scalar1=256, scalar2=None,
                op0=mybir.AluOpType.mult)
            nc.vector.tensor_tensor(
                out=idx_b[:], in0=idx_b[:], in1=lo32[:],
                op=mybir.AluOpType.add)

            # accD[dst, feat] = segment-sum of gathered src rows for this
            # block, accumulated in PSUM across C edge chunks.
            accD = ps1.tile([P, D], f32, space="PSUM")
            for k in range(C):
                gq = gpool.tile([P, D + 1], i8)
                nc.gpsimd.indirect_dma_start(
                    out=gq[:], out_offset=None, in_=hq_gat[:],
                    in_offset=bass.IndirectOffsetOnAxis(ap=idx_b[:, k:k + 1], axis=0),
                )
                # dequantize: s = 2^((e8-192)/16) from the row's scale byte
                gs = gpool.tile([P, 1], f32)
                nc.vector.tensor_copy(out=gs[:], in_=gq[:, D:D + 1])
                nc.vector.tensor_scalar_add(out=gs[:], in0=gs[:], scalar1=-192.0)
                nc.scalar.activation(out=gs[:], in_=gs[:],
                                     func=mybir.ActivationFunctionType.Exp,
                                     scale=0.04332169878499658)
                g_t = gpool.tile([P, D], bf16)
                nc.vector.tensor_scalar_mul(out=g_t[:], in0=gq[:, 0:D], scalar1=gs[:])
                # S_T[e, j] = (rel[e] == j); padded edges have rel=-1 -> all 0
                s_t = spool.tile([P, P], bf16)
                nc.vector.tensor_tensor(
                    out=s_t[:],
                    in0=e8[:, 2 * C + k:2 * C + k + 1].to_broadcast([P, P]),
                    in1=iota_t[:],
                    op=mybir.AluOpType.is_equal,
                )
                nc.tensor.matmul(
                    out=accD[:],
                    lhsT=s_t[:],
                    rhs=g_t[:],
                    start=(k == 0),
                    stop=(k == C - 1),
                )

            accS = apool.tile([P, D], f32)
            nc.vector.tensor_copy(out=accS[:], in_=accD[:])

            # out_ps[dst, :] = sum_kc A_kc @ W_kc (transpose chunks for lhsT)
            out_ps = ps2.tile([P, D], f32, space="PSUM")
            for kc in range(KC):
                tps = pst.tile([P, P], f32, space="PSUM")
                nc.tensor.transpose(
                    out=tps[:], in_=accS[:, kc * P:(kc + 1) * P],
                    identity=ident_t[:])
                lhsT_kc = tpool.tile([P, P], bf16)
                nc.vector.tensor_copy(out=lhsT_kc[:], in_=tps[:])
                nc.tensor.matmul(
                    out=out_ps[:],
                    lhsT=lhsT_kc[:],
                    rhs=w_t[:, kc * D:(kc + 1) * D],
                    start=(kc == 0),
                    stop=(kc == KC - 1),
                )
            # int8-quantize agg@W directly: per-row scale is relative, so
            # the dst-norm multiply and bias add fold exactly into the
            # host-side dequant. The scale ships as one u8 exponent byte
            # e = RN(16*log2(rmax)+129.5) (so decoded s' >= rmax); device
            # and host both decode s' = 2^((e-128)/16), keeping dequant
            # consistent up to the Exp LUT's tiny approximation error.
            out_f = opool.tile([P, D], f32)
            nc.vector.tensor_copy(out=out_f[:], in_=out_ps[:])
            rmax = opool.tile([P, 1], f32)
            nc.vector.tensor_reduce(
                out=rmax[:], in_=out_f[:], axis=mybir.AxisListType.X,
                op=mybir.AluOpType.max, apply_absolute_value=True)
            nc.vector.tensor_scalar_max(out=rmax[:], in0=rmax[:], scalar1=1e-20)
            kf = opool.tile([P, 1], f32)
            nc.scalar.activation(out=kf[:], in_=rmax[:],
                                 func=mybir.ActivationFunctionType.Ln)
            e8 = opool.tile([P, 1], u8)
            nc.vector.tensor_scalar(
                out=e8[:], in0=kf[:], scalar1=23.083120654223414,
                scalar2=129.5, op0=mybir.AluOpType.mult,
                op1=mybir.AluOpType.add)
            ef = opool.tile([P, 1], f32)
            nc.vector.tensor_copy(out=ef[:], in_=e8[:])
            nc.vector.tensor_scalar_add(out=ef[:], in0=ef[:], scalar1=-128.0)
            rinv = opool.tile([P, 1], f32)
            nc.scalar.activation(out=rinv[:], in_=ef[:],
                                 func=mybir.ActivationFunctionType.Exp,
                                 scale=-0.04332169878499658)
            v_t = opool.tile([P, D], f32)
            nc.vector.tensor_tensor(
                out=v_t[:], in0=out_f[:],
                in1=rinv[:].to_broadcast([P, D]),
                op=mybir.AluOpType.mult,
            )
            # HW f32->u8 cast is round-to-nearest-even with saturation, so a
            # plain +128 offset gives ideal symmetric rounding. (CoreSim
            # truncates instead, inflating sim-reported error only.)
            q_t = opool.tile([P, D], u8)
            nc.vector.tensor_scalar(
                out=q_t[:], in0=v_t[:], scalar1=127.0, scalar2=128.0,
                op0=mybir.AluOpType.mult, op1=mybir.AluOpType.add,
            )
            nc.sync.dma_start(out=out[b * P:(b + 1) * P, 0:D], in_=q_t[:])
            nc.sync.dma_start(out=out[b * P:(b + 1) * P, D:D + 1], in_=e8[:])
    nc.compile()
    return nc


def _prep(h, norm, weight, bias, src, dst):
    # per-core upload rows: 0..NPC-1 node (h*norm) rows, NPC..SROWS-1 W rows;
    # everything int8 with exactly-invertible per-row f32 scales.
    vals = np.zeros((NCORES, SROWS, D), dtype=np.float32)
    hn = h * norm
    for c in range(NCORES):
        lo = c * NPC
        hi = min(N, lo + NPC)
        if lo < N:
            vals[c, :hi - lo, :] = hn[lo:hi]
    vals[:, NPC:, :] = weight.astype(np.float32).reshape(NCORES, WPC, D)

    flat = vals.reshape(NCORES * SROWS, D)
    s = np.abs(flat).max(axis=1, keepdims=True)
    s /= 127.0
    np.maximum(s, 1e-30, out=s)
    # pow2 scale, one exponent byte: s' = 2^((e-192)/16) >= s, e in [-128,127]
    e8 = np.clip(np.ceil(np.log2(s) * 16.0) + 192.0, -128, 127)
    sq = np.exp2((e8 - 192.0) / 16.0).astype(np.float32)
    np.multiply(flat, 1.0 / sq, out=flat)
    np.rint(flat, out=flat)
    np.clip(flat, -127, 127, out=flat)
    q8 = np.empty((NCORES * SROWS, D + 1), dtype=np.int8)
    q8[:, :D] = flat.astype(np.int8)
    q8[:, D:] = e8.astype(np.int8)
    q = q8.reshape(NCORES, SROWS, D + 1)

    src = np.asarray(src, dtype=np.int64)
    dst = np.asarray(dst, dtype=np.int64)
    core_of = dst // NPC
    blk_of = (dst % NPC) // P
    # node n lives at gathered row (n // NPC) * SROWS + (n % NPC)
    src = (src // NPC) * SROWS + (src % NPC)

    # chunk count: max edges landing in any (core, block), ceil to 128
    counts = np.zeros((NCORES, NBLK), dtype=np.int64)
    np.add.at(counts, (core_of, blk_of), 1)
    C = max(1, int(-(-counts.max() // P)))

    edg_all = np.zeros((NCORES, NBLK, P, 3 * C), dtype=np.uint8)
    edg_all[:, :, :, 2 * C:] = 255
    gkey = core_of * NBLK + blk_of
    order = np.argsort(gkey, kind="stable")
    s_sorted = src[order]
    d_sorted = dst[order]
    g_sorted = gkey[order]
    starts = np.searchsorted(g_sorted, np.arange(NCORES * NBLK))
    rank = np.arange(len(g_sorted)) - starts[g_sorted]
    cc, bb, pp, kk = (g_sorted // NBLK, g_sorted % NBLK, rank % P, rank // P)
    edg_all[cc, bb, pp, kk] = s_sorted & 0xFF
    edg_all[cc, bb, pp, C + kk] = s_sorted >> 8
    edg_all[cc, bb, pp, 2 * C + kk] = d_sorted % P

    in_maps = []
    for c in range(NCORES):
        in_maps.append({
            "hq": q[c],
            "edg": edg_all[c],
        })
    return C, in_maps


def _unpack(res, norm, bias):
    """Dequantize (uint8 q | u8 scale-exponent) rows, applying dst-norm and
    bias: out = (q-128) * 2^((e-128)/16) / 127 * norm + bias."""
    normv = np.zeros((NPAD, 1), dtype=np.float32)
    normv[:N] = norm
    bias = bias.astype(np.float32)[None, :]
    outs = []
    for c in range(NCORES):
        raw = np.asarray(res[c]["out"])
        q = raw[:, :D].astype(np.float32)
        e = raw[:, D:D + 1].astype(np.float32)
        s = np.exp2((e - 128.0) / 16.0).astype(np.float32)
        sc = normv[c * NPC:(c + 1) * NPC] * (s / 127.0)
        outs.append((q - 128.0) * sc + bias)
    return np.concatenate(outs, axis=0)[:N]


_NC_CACHE = {}


def kernel(h, norm, weight, bias, src, dst):
    h = np.asarray(h, dtype=np.float32)
    norm = np.asarray(norm, dtype=np.float32)
    weight = np.asarray(weight, dtype=np.float32)
    bias = np.asarray(bias, dtype=np.float32)
    C, in_maps = _prep(h, norm, weight, bias, src, dst)
    nc = _NC_CACHE.get(C)
    if nc is None:
        nc = _NC_CACHE[C] = _build(C)
    res = run_bass_kernel_spmd(nc, in_maps, list(range(NCORES))).results
    return _unpack(res, norm, bias)
